# revision 14
# baseline (speedup 1.0000x reference)
"""Trainium2 Bass kernel for nn_C4TransformerVM (neural-ALU 4-byte one-hot adder).

The reference module performs 32-bit addition over one-hot byte encodings via
LUT matmuls + softmax(TEMP=100). With exact one-hot inputs the softmax output
is one-hot to within ~4e-44 (fp32 subnormals), so an exact
decode -> integer ripple-carry add -> one-hot encode pipeline matches the
reference to ~1e-43 relative error while touching each input/output element
exactly once (memory-bound regime).

Per core (pure data parallel over the batch, 4096 tokens each; production
variant "v10s" (= v9 + half-tile stores: each byte's 1MB store is issued as
two 512KB halves so the first fires 4 encode-chunks earlier) with all-ACT
encode, ~152us/core. Roofline: per-NC HBM bandwidth is ~342 GB/s with reads
and writes SHARING that budget (measured reads-only 98us/32MB, writes-only
49us/16MB, combined 152us/48MB), so the 48MB-of-I/O floor is ~147us
arithmetic / ~144.5us best measured DMA-only structure; the kernel runs at
~95% of that. DVE (decode dots + carry, ~106us/rep) paces load-buffer
release, which is why the encode is kept entirely off DVE):
  pipeline: per 1024-token group, the four bytes are processed as a rippled
          byte pipeline (load_i -> dot_i -> carry-step_i -> encode_i ->
          store_i), so the first store launches ~10us into the group instead
          of after all four byte loads, keeping the store ring busy through
          the per-iteration engine-barrier refill
  decode: fused multiply-by-iota + sum-reduce (scalar_tensor_tensor
          accum_out) directly on each loaded one-hot tile (DVE; one ALU pass
          per input element, which is the floor) -> exact fp32 byte values
  carry:  batched ripple-carry on [128, G] value tiles (DVE, tiny)
  encode: d = |iota - r| via ACT Abs with per-partition bias into PSUM, then
          one-hot = Relu(1-d) on ACT (all chunks; ACT total ~129us/rep still
          fits under the DMA floor, and unloading DVE bought ~3us/rep)
  stores: issued from GPSIMD's SWDGE ring so they cannot head-of-line block
          loads on the sync-engine HWDGE ring (only SP/ACT/GPSIMD can issue
          DMAs; GPSIMD tensor compute is pathologically slow - DMA only)
  layout: tokens -> (partition, chunk) with consecutive tokens per partition,
          so every 1MB DMA is 8KB-contiguous per partition
Rejected with measurements: 2MB/4MB DMAs (no change), loads spread over 2-3
DGE rings (worse), single/quarter stores (worse), scalar-ring stores (equal),
deeper buffer rings (worse), barrier loop reset (worse).
"""

from contextlib import ExitStack

import numpy as np

import concourse.bacc as bacc
import concourse.tile as tile
from concourse import mybir
from concourse.bass_utils import run_bass_kernel_spmd

N_CORES = 8
N_TOK = 32768
P = 128
TOK_PER_CORE = N_TOK // N_CORES  # 4096


def build_nc(
    n_tok_core=TOK_PER_CORE,
    g_blocks=8,
    n_cores=N_CORES,
    n_reps=None,
    variant="full",
    scratch_io=False,
    bufs=None,
    store_engine=None,
    encode_dve_every=4,
    staggered=True,
):
    """Build + compile the per-core Bass program (SPMD: same on all cores).

    n_reps: if set, wrap the whole body in a runtime For_i loop that re-executes
    it n_reps times (for on-device timing via wall-clock differencing).
    variant: "full" | "dmaonly" (loads+stores, no compute) | "noenc"
    (decode+carry but store s instead of encoding) | "acte" (encode on ACT).
    """
    pool_hint = variant == "v10p"
    if pool_hint:
        variant = "v10s"
    nb = n_tok_core // P  # 128-token blocks per core
    g = min(g_blocks, nb)  # blocks per pipeline group
    assert nb % g == 0

    f32 = mybir.dt.float32
    op = mybir.AluOpType

    nc = bacc.Bacc(
        "TRN2",
        target_bir_lowering=False,
        debug=False,
        num_devices=n_cores,
    )
    _store_eng = lambda: {  # noqa: E731 - resolved after nc exists
        None: nc.gpsimd,
        "gpsimd": nc.gpsimd,
        "scalar": nc.scalar,
        "sync": nc.sync,
    }[store_engine]
    if scratch_io:
        # timing-only build: full-size DRAM buffers are device scratch so each
        # exec ships only iota in / a tiny tile out over the host tunnel
        a_d = nc.dram_tensor("a", [4, n_tok_core, 256], f32, kind="Internal").ap()
        b_d = nc.dram_tensor("b", [4, n_tok_core, 256], f32, kind="Internal").ap()
        o_d = nc.dram_tensor("o", [4, n_tok_core, 256], f32, kind="Internal").ap()
        t_d = nc.dram_tensor("t", [P, 256], f32, kind="ExternalOutput").ap()
    else:
        a_d = nc.dram_tensor("a", [4, n_tok_core, 256], f32, kind="ExternalInput").ap()
        b_d = nc.dram_tensor("b", [4, n_tok_core, 256], f32, kind="ExternalInput").ap()
        o_d = nc.dram_tensor("o", [4, n_tok_core, 256], f32, kind="ExternalOutput").ap()
    iota_d = nc.dram_tensor("iota", [P, 256], f32, kind="ExternalInput").ap()

    # scale buffer counts down for bigger tiles so SBUF (~24MB) still fits
    io_b, work_b, enc_b = (3, 2, 3) if g >= 16 else (6, 3, 4)
    if variant.startswith("v8"):
        io_b, work_b, enc_b = (7, 2, 5) if g == 8 else (12, 2, 8)
    if bufs is not None:
        io_b, work_b, enc_b = bufs
    with tile.TileContext(nc) as tc, ExitStack() as ctx:
        const = ctx.enter_context(tc.tile_pool(name="const", bufs=1))
        io = ctx.enter_context(tc.tile_pool(name="io", bufs=io_b))
        work = ctx.enter_context(tc.tile_pool(name="work", bufs=work_b))
        vals = ctx.enter_context(tc.tile_pool(name="vals", bufs=3))
        enc = ctx.enter_context(tc.tile_pool(name="enc", bufs=enc_b))
        psum = ctx.enter_context(tc.tile_pool(name="psum", bufs=6, space="PSUM"))

        iota_t = const.tile([P, 256], f32)
        nc.sync.dma_start(iota_t[:], iota_d)
        if scratch_io:
            nc.sync.dma_start(t_d, iota_t[:])

        rep_ctx = (
            tc.For_i(
                0,
                n_reps,
                1,
                staggered_reset=staggered,
                hint_engines=(
                    mybir.EngineType.DVE,
                    mybir.EngineType.Activation,
                    mybir.EngineType.SP,
                )
                + ((mybir.EngineType.Pool,) if pool_hint else ()),
            )
            if n_reps is not None
            else None
        )
        if rep_ctx is not None:
            ctx.enter_context(rep_ctx)

        for gi in range(nb // g):
            tok0 = gi * g * P
            ntok_g = g * P

            if variant in ("dmar", "dmaw"):
                # directional probes: dmar = loads only, dmaw = stores only
                for i in range(4):
                    if variant == "dmar":
                        for src, tag in ((a_d, "a"), (b_d, "b")):
                            v = src[i, tok0 : tok0 + ntok_g, :].rearrange(
                                "(p c) f -> p c f", p=P
                            )
                            t_ = io.tile([P, g, 256], f32, tag=tag)
                            nc.sync.dma_start(t_[:], v)
                    else:
                        o_t = enc.tile([P, g, 256], f32, tag="o")
                        nc.vector.memset(o_t[:], 0.0)
                        o_v = o_d[i, tok0 : tok0 + ntok_g, :].rearrange(
                            "(p c) f -> p c f", p=P
                        )
                        nc.gpsimd.dma_start(o_v, o_t[:])
                continue

            if variant.startswith("dma"):
                bf16 = mybir.dt.bfloat16
                if variant == "dma2":
                    load_engs, store_engs = (nc.sync, nc.sync), (nc.scalar,)
                elif variant == "dma3":
                    load_engs, store_engs = (nc.sync, nc.sync), (nc.gpsimd,)
                elif variant == "dma4":
                    load_engs, store_engs = (nc.sync, nc.scalar), (nc.gpsimd,)
                elif variant == "dma5":
                    load_engs, store_engs = (nc.sync, nc.scalar, nc.gpsimd), (
                        nc.scalar,
                        nc.gpsimd,
                        nc.sync,
                    )
                elif variant in ("dmabf", "dmab2"):
                    load_engs, store_engs = (nc.sync, nc.sync), (nc.gpsimd,)
                else:
                    load_engs, store_engs = (nc.sync, nc.sync), (nc.sync,)
                strided = variant in ("dmabf", "dmab2")
                # [4, n, 256] f32 viewed as [4, n, 256, 2] bf16; [..., 1] = upper halves
                if strided:
                    a_hi = a_d.bitcast(bf16).rearrange("i n (f two) -> i n f two", two=2)
                    b_hi = b_d.bitcast(bf16).rearrange("i n (f two) -> i n f two", two=2)
                    o_hi = o_d.bitcast(bf16).rearrange("i n (f two) -> i n f two", two=2)
                for i in range(4):
                    if strided:
                        a_v = a_hi[i, tok0 : tok0 + ntok_g, :, 1].rearrange(
                            "(p c) f -> p c f", p=P
                        )
                        b_v = b_hi[i, tok0 : tok0 + ntok_g, :, 1].rearrange(
                            "(p c) f -> p c f", p=P
                        )
                        a_t = io.tile([P, g, 256], bf16, tag="a")
                        b_t = io.tile([P, g, 256], bf16, tag="b")
                    else:
                        a_v = a_d[i, tok0 : tok0 + ntok_g, :].rearrange(
                            "(p c) f -> p c f", p=P
                        )
                        b_v = b_d[i, tok0 : tok0 + ntok_g, :].rearrange(
                            "(p c) f -> p c f", p=P
                        )
                        a_t = io.tile([P, g, 256], f32, tag="a")
                        b_t = io.tile([P, g, 256], f32, tag="b")
                    load_engs[(2 * i) % len(load_engs)].dma_start(a_t[:], a_v)
                    load_engs[(2 * i + 1) % len(load_engs)].dma_start(b_t[:], b_v)
                    st = store_engs[i % len(store_engs)]
                    if variant == "dmab2":
                        # strided upper-half store (8MiB instead of 16MiB)
                        o_v = o_hi[i, tok0 : tok0 + ntok_g, :, 1].rearrange(
                            "(p c) f -> p c f", p=P
                        )
                        st.dma_start(o_v, a_t[:])
                    elif variant == "dmabf":
                        o_t = enc.tile([P, g, 256], f32, tag="o")
                        nc.vector.memset(o_t[:], 0.0)
                        o_v = o_d[i, tok0 : tok0 + ntok_g, :].rearrange(
                            "(p c) f -> p c f", p=P
                        )
                        st.dma_start(o_v, o_t[:])
                    else:
                        o_v = o_d[i, tok0 : tok0 + ntok_g, :].rearrange(
                            "(p c) f -> p c f", p=P
                        )
                        st.dma_start(o_v, a_t[:])
                continue

            if variant in ("dcmp", "denc"):
                # probes: dma3 traffic + one engine's compute, no cross deps
                ta = vals.tile([P, 4, g], f32, tag="ta")
                trash_d = work.tile([P, 256], f32, tag="trash_d")
                for i in range(4):
                    a_v = a_d[i, tok0 : tok0 + ntok_g, :].rearrange(
                        "(p c) f -> p c f", p=P
                    )
                    b_v = b_d[i, tok0 : tok0 + ntok_g, :].rearrange(
                        "(p c) f -> p c f", p=P
                    )
                    a_t = io.tile([P, g, 256], f32, tag="a")
                    b_t = io.tile([P, g, 256], f32, tag="b")
                    nc.sync.dma_start(a_t[:], a_v)
                    nc.sync.dma_start(b_t[:], b_v)
                    if variant == "dcmp":
                        for c in range(g):
                            for src in (a_t, b_t):
                                nc.vector.scalar_tensor_tensor(
                                    out=trash_d[:],
                                    in0=src[:, c, :],
                                    scalar=1.0,
                                    in1=iota_t[:],
                                    op0=op.mult,
                                    op1=op.mult,
                                    accum_out=ta[:, i, c % 4 : c % 4 + 1],
                                )
                        st_t = a_t
                    else:
                        o_t = enc.tile([P, g, 256], f32, tag="o")
                        for c in range(g):
                            idx = i * g + c
                            r_ap = iota_t[:, i : i + 1]
                            if idx % 4 != 3:
                                d_t = psum.tile([P, 256], f32, tag="d")
                                nc.scalar.activation(
                                    out=d_t[:],
                                    in_=iota_t[:],
                                    func=mybir.ActivationFunctionType.Abs,
                                    bias=r_ap,
                                    scale=-1.0,
                                )
                                nc.scalar.activation(
                                    out=o_t[:, c, :],
                                    in_=d_t[:],
                                    func=mybir.ActivationFunctionType.Relu,
                                    bias=1.0,
                                    scale=-1.0,
                                )
                            else:
                                nc.vector.tensor_scalar(
                                    out=o_t[:, c, :],
                                    in0=iota_t[:],
                                    scalar1=r_ap,
                                    scalar2=None,
                                    op0=op.is_equal,
                                )
                        st_t = o_t
                    o_v = o_d[i, tok0 : tok0 + ntok_g, :].rearrange(
                        "(p c) f -> p c f", p=P
                    )
                    _store_eng().dma_start(o_v, st_t[:])
                continue

            if variant in ("v10", "v10s", "v10q"):
                # v9 + split stores: each byte's store is issued in pieces as
                # soon as the covering encode chunks are done (v10s halves,
                # v10q quarters); v10 also splits loads (measured worse)
                split_loads = variant == "v10"
                n_pieces = 4 if variant == "v10q" else 2
                piece = g // n_pieces
                h = g // 2
                trash_d = work.tile([P, 256], f32, tag="trash_d")
                prev_c = None
                for i in range(4):
                    a_v = a_d[i, tok0 : tok0 + ntok_g, :].rearrange(
                        "(p c) f -> p c f", p=P
                    )
                    b_v = b_d[i, tok0 : tok0 + ntok_g, :].rearrange(
                        "(p c) f -> p c f", p=P
                    )
                    a_t = io.tile([P, g, 256], f32, tag="a")
                    b_t = io.tile([P, g, 256], f32, tag="b")
                    if split_loads:
                        for t_, v_ in ((a_t, a_v), (b_t, b_v)):
                            nc.sync.dma_start(t_[:, :h, :], v_[:, :h, :])
                            nc.sync.dma_start(t_[:, h:, :], v_[:, h:, :])
                    else:
                        nc.sync.dma_start(a_t[:], a_v)
                        nc.sync.dma_start(b_t[:], b_v)
                    ta = vals.tile([P, g], f32, tag="ta")
                    tb = vals.tile([P, g], f32, tag="tb")
                    for c in range(g):
                        for src, dst in ((a_t, ta), (b_t, tb)):
                            nc.vector.scalar_tensor_tensor(
                                out=trash_d[:],
                                in0=src[:, c, :],
                                scalar=1.0,
                                in1=iota_t[:],
                                op0=op.mult,
                                op1=op.mult,
                                accum_out=dst[:, c : c + 1],
                            )
                    t_i = vals.tile([P, g], f32, tag="t")
                    nc.vector.tensor_add(t_i[:], ta[:], tb[:])
                    if prev_c is not None:
                        nc.vector.tensor_add(t_i[:], t_i[:], prev_c)
                    c_i = vals.tile([P, g], f32, tag="c")
                    nc.vector.tensor_scalar(
                        out=c_i[:],
                        in0=t_i[:],
                        scalar1=255.5,
                        scalar2=None,
                        op0=op.is_gt,
                    )
                    r_i = vals.tile([P, g], f32, tag="r")
                    nc.vector.scalar_tensor_tensor(
                        out=r_i[:],
                        in0=c_i[:],
                        scalar=-256.0,
                        in1=t_i[:],
                        op0=op.mult,
                        op1=op.add,
                    )
                    prev_c = c_i[:]

                    o_t = enc.tile([P, g, 256], f32, tag="o")
                    o_v = o_d[i, tok0 : tok0 + ntok_g, :].rearrange(
                        "(p c) f -> p c f", p=P
                    )
                    for c in range(g):
                        r_ap = r_i[:, c : c + 1]
                        if (
                            encode_dve_every is None
                            or c % encode_dve_every != encode_dve_every - 1
                        ):
                            d_t = psum.tile([P, 256], f32, tag="d")
                            nc.scalar.activation(
                                out=d_t[:],
                                in_=iota_t[:],
                                func=mybir.ActivationFunctionType.Abs,
                                bias=r_ap,
                                scale=-1.0,
                            )
                            nc.scalar.activation(
                                out=o_t[:, c, :],
                                in_=d_t[:],
                                func=mybir.ActivationFunctionType.Relu,
                                bias=1.0,
                                scale=-1.0,
                            )
                        else:
                            nc.vector.tensor_scalar(
                                out=o_t[:, c, :],
                                in0=iota_t[:],
                                scalar1=r_ap,
                                scalar2=None,
                                op0=op.is_equal,
                            )
                        if (c + 1) % piece == 0 and c != g - 1:
                            lo_c = (c + 1) - piece
                            _store_eng().dma_start(
                                o_v[:, lo_c : c + 1, :], o_t[:, lo_c : c + 1, :]
                            )
                    _store_eng().dma_start(
                        o_v[:, g - piece :, :], o_t[:, g - piece :, :]
                    )
                continue

            if variant == "v9":
                # byte-interleaved pipeline: each byte's store launches as soon
                # as its own loads + dots + carry step + encode finish, so the
                # store ring starts ~10us into each group instead of ~30us
                trash_d = work.tile([P, 256], f32, tag="trash_d")
                prev_c = None
                for i in range(4):
                    a_v = a_d[i, tok0 : tok0 + ntok_g, :].rearrange(
                        "(p c) f -> p c f", p=P
                    )
                    b_v = b_d[i, tok0 : tok0 + ntok_g, :].rearrange(
                        "(p c) f -> p c f", p=P
                    )
                    a_t = io.tile([P, g, 256], f32, tag="a")
                    b_t = io.tile([P, g, 256], f32, tag="b")
                    nc.sync.dma_start(a_t[:], a_v)
                    nc.sync.dma_start(b_t[:], b_v)
                    ta = vals.tile([P, g], f32, tag="ta")
                    tb = vals.tile([P, g], f32, tag="tb")
                    for c in range(g):
                        nc.vector.scalar_tensor_tensor(
                            out=trash_d[:],
                            in0=a_t[:, c, :],
                            scalar=1.0,
                            in1=iota_t[:],
                            op0=op.mult,
                            op1=op.mult,
                            accum_out=ta[:, c : c + 1],
                        )
                        nc.vector.scalar_tensor_tensor(
                            out=trash_d[:],
                            in0=b_t[:, c, :],
                            scalar=1.0,
                            in1=iota_t[:],
                            op0=op.mult,
                            op1=op.mult,
                            accum_out=tb[:, c : c + 1],
                        )
                    t_i = vals.tile([P, g], f32, tag="t")
                    nc.vector.tensor_add(t_i[:], ta[:], tb[:])
                    if prev_c is not None:
                        nc.vector.tensor_add(t_i[:], t_i[:], prev_c)
                    c_i = vals.tile([P, g], f32, tag="c")
                    nc.vector.tensor_scalar(
                        out=c_i[:],
                        in0=t_i[:],
                        scalar1=255.5,
                        scalar2=None,
                        op0=op.is_gt,
                    )
                    r_i = vals.tile([P, g], f32, tag="r")
                    nc.vector.scalar_tensor_tensor(
                        out=r_i[:],
                        in0=c_i[:],
                        scalar=-256.0,
                        in1=t_i[:],
                        op0=op.mult,
                        op1=op.add,
                    )
                    prev_c = c_i[:]

                    o_t = enc.tile([P, g, 256], f32, tag="o")
                    for c in range(g):
                        r_ap = r_i[:, c : c + 1]
                        if c % 4 != 3:
                            d_t = psum.tile([P, 256], f32, tag="d")
                            nc.scalar.activation(
                                out=d_t[:],
                                in_=iota_t[:],
                                func=mybir.ActivationFunctionType.Abs,
                                bias=r_ap,
                                scale=-1.0,
                            )
                            nc.scalar.activation(
                                out=o_t[:, c, :],
                                in_=d_t[:],
                                func=mybir.ActivationFunctionType.Relu,
                                bias=1.0,
                                scale=-1.0,
                            )
                        else:
                            nc.vector.tensor_scalar(
                                out=o_t[:, c, :],
                                in0=iota_t[:],
                                scalar1=r_ap,
                                scalar2=None,
                                op0=op.is_equal,
                            )
                    o_v = o_d[i, tok0 : tok0 + ntok_g, :].rearrange(
                        "(p c) f -> p c f", p=P
                    )
                    _store_eng().dma_start(o_v, o_t[:])
                continue

            if variant.startswith("v8"):
                # decode: one fused mult+reduce pass per input element, split
                # DVE/GPSIMD; encode: one-hot via is_equal (DVE/GPS) or
                # Abs->Relu through PSUM (ACT); stores from gpsimd SWDGE
                store_eng = nc.scalar if variant == "v8s" else nc.gpsimd
                ta = vals.tile([P, 4, g], f32, tag="ta")
                tb = vals.tile([P, 4, g], f32, tag="tb")
                trash_d = work.tile([P, 256], f32, tag="trash_d")
                trash_g = work.tile([P, 256], f32, tag="trash_g")
                for i in range(4):
                    a_v = a_d[i, tok0 : tok0 + ntok_g, :].rearrange(
                        "(p c) f -> p c f", p=P
                    )
                    b_v = b_d[i, tok0 : tok0 + ntok_g, :].rearrange(
                        "(p c) f -> p c f", p=P
                    )
                    a_t = io.tile([P, g, 256], f32, tag="a")
                    b_t = io.tile([P, g, 256], f32, tag="b")
                    nc.sync.dma_start(a_t[:], a_v)
                    nc.sync.dma_start(b_t[:], b_v)
                    for c in range(g):
                        for src, dst in ((a_t, ta), (b_t, tb)):
                            nc.vector.scalar_tensor_tensor(
                                out=trash_d[:],
                                in0=src[:, c, :],
                                scalar=1.0,
                                in1=iota_t[:],
                                op0=op.mult,
                                op1=op.mult,
                                accum_out=dst[:, i, c : c + 1],
                            )
                t0 = vals.tile([P, 4, g], f32, tag="t0")
                nc.vector.tensor_add(t0[:], ta[:], tb[:])

                # ripple carry (all small [P, g] DVE ops)
                r = vals.tile([P, 4, g], f32, tag="r")
                c_t = vals.tile([P, 4, g], f32, tag="c")
                prev = None
                for i in range(4):
                    if prev is None:
                        t_i = t0[:, i, :]
                    else:
                        t_tile = vals.tile([P, g], f32, tag="t")
                        nc.vector.tensor_add(t_tile[:], t0[:, i, :], prev)
                        t_i = t_tile[:]
                    nc.vector.tensor_scalar(
                        out=c_t[:, i, :],
                        in0=t_i,
                        scalar1=255.5,
                        scalar2=None,
                        op0=op.is_gt,
                    )
                    nc.vector.scalar_tensor_tensor(
                        out=r[:, i, :],
                        in0=c_t[:, i, :],
                        scalar=-256.0,
                        in1=t_i,
                        op0=op.mult,
                        op1=op.add,
                    )
                    prev = c_t[:, i, :]

                for i in range(4):
                    o_t = enc.tile([P, g, 256], f32, tag="o")
                    for c in range(g):
                        idx = i * g + c
                        r_ap = r[:, i, c : c + 1]
                        if idx % 4 != 3:
                            d_t = psum.tile([P, 256], f32, tag="d")
                            nc.scalar.activation(
                                out=d_t[:],
                                in_=iota_t[:],
                                func=mybir.ActivationFunctionType.Abs,
                                bias=r_ap,
                                scale=-1.0,
                            )
                            nc.scalar.activation(
                                out=o_t[:, c, :],
                                in_=d_t[:],
                                func=mybir.ActivationFunctionType.Relu,
                                bias=1.0,
                                scale=-1.0,
                            )
                        else:
                            nc.vector.tensor_scalar(
                                out=o_t[:, c, :],
                                in0=iota_t[:],
                                scalar1=r_ap,
                                scalar2=None,
                                op0=op.is_equal,
                            )
                    o_v = o_d[i, tok0 : tok0 + ntok_g, :].rearrange(
                        "(p c) f -> p c f", p=P
                    )
                    store_eng.dma_start(o_v, o_t[:])
                continue

            # t0[:, i, c] = av + bv for byte i, block c (value 0..510)
            t0 = vals.tile([P, 4, g], f32, tag="t0")
            for i in range(4):
                a_v = a_d[i, tok0 : tok0 + ntok_g, :].rearrange(
                    "(p c) f -> p c f", p=P
                )
                b_v = b_d[i, tok0 : tok0 + ntok_g, :].rearrange(
                    "(p c) f -> p c f", p=P
                )
                a_t = io.tile([P, g, 256], f32, tag="a")
                b_t = io.tile([P, g, 256], f32, tag="b")
                nc.sync.dma_start(a_t[:], a_v)
                nc.sync.dma_start(b_t[:], b_v)
                s_t = work.tile([P, g, 256], f32, tag="s")
                add_eng = nc.gpsimd if variant == "v3" else nc.vector
                add_eng.tensor_add(s_t[:], a_t[:], b_t[:])
                trash = work.tile([P, 256], f32, tag="trash")
                for c in range(g):
                    nc.vector.scalar_tensor_tensor(
                        out=trash[:],
                        in0=s_t[:, c, :],
                        scalar=1.0,
                        in1=iota_t[:],
                        op0=op.mult,
                        op1=op.mult,
                        accum_out=t0[:, i, c : c + 1],
                    )
                if variant == "noenc":
                    o_v = o_d[i, tok0 : tok0 + ntok_g, :].rearrange(
                        "(p c) f -> p c f", p=P
                    )
                    nc.sync.dma_start(o_v, s_t[:])

            if variant == "noenc":
                continue

            # ripple carry: t_i = t0_i + c_i ; c_{i+1} = t_i > 255 ; r_i = t_i - 256*c_{i+1}
            r = vals.tile([P, 4, g], f32, tag="r")
            c_t = vals.tile([P, 4, g], f32, tag="c")
            prev = None
            for i in range(4):
                if prev is None:
                    t_i = t0[:, i, :]
                else:
                    t_tile = vals.tile([P, g], f32, tag="t")
                    nc.vector.tensor_add(t_tile[:], t0[:, i, :], prev)
                    t_i = t_tile[:]
                nc.vector.tensor_scalar(
                    out=c_t[:, i, :], in0=t_i, scalar1=255.5, scalar2=None, op0=op.is_gt
                )
                if variant not in ("full",):
                    # rn = 256*c - t = -r  (bias for ACT-side |iota - r|)
                    nc.vector.scalar_tensor_tensor(
                        out=r[:, i, :],
                        in0=c_t[:, i, :],
                        scalar=256.0,
                        in1=t_i,
                        op0=op.mult,
                        op1=op.subtract,
                    )
                else:
                    nc.vector.scalar_tensor_tensor(
                        out=r[:, i, :],
                        in0=c_t[:, i, :],
                        scalar=-256.0,
                        in1=t_i,
                        op0=op.mult,
                        op1=op.add,
                    )
                prev = c_t[:, i, :]

            for i in range(4):
                o_t = enc.tile([P, g, 256], f32, tag="o")
                if variant not in ("full",):
                    for c in range(g):
                        d_t = psum.tile([P, 256], f32, tag="d")
                        nc.scalar.activation(
                            out=d_t[:],
                            in_=iota_t[:],
                            func=mybir.ActivationFunctionType.Abs,
                            bias=r[:, i, c : c + 1],
                            scale=1.0,
                        )
                        if (
                            variant == "v2"
                            or (variant in ("v4", "v6", "v7") and c % 4 == 0)
                            or (variant == "v6b" and c % 2 == 0)
                        ):
                            nc.vector.tensor_scalar(
                                out=o_t[:, c, :],
                                in0=d_t[:],
                                scalar1=0.5,
                                scalar2=None,
                                op0=op.is_lt,
                            )
                        else:
                            nc.scalar.activation(
                                out=o_t[:, c, :],
                                in_=d_t[:],
                                func=mybir.ActivationFunctionType.Relu,
                                bias=1.0,
                                scale=-1.0,
                            )
                else:
                    for c in range(g):
                        nc.gpsimd.tensor_scalar(
                            out=o_t[:, c, :],
                            in0=iota_t[:],
                            scalar1=r[:, i, c : c + 1],
                            scalar2=None,
                            op0=op.is_equal,
                        )
                o_v = o_d[i, tok0 : tok0 + ntok_g, :].rearrange(
                    "(p c) f -> p c f", p=P
                )
                store_eng = (
                    nc.gpsimd
                    if variant in ("v5", "v6", "v6b")
                    else (nc.scalar if variant == "v7" else nc.sync)
                )
                store_eng.dma_start(o_v, o_t[:])

    nc.compile()
    return nc


_NC_CACHE = {}

# production build configuration (test.py's device timer uses the same dict so
# the printed HW exec time is measured on the identical per-rep body).
# encode_dve_every=None = all-ACT encode: keeping the encode off DVE leaves DVE
# (the load-buffer-release pacer: decode dots + carry) ~3us/rep more slack for
# load/store overlap — measured 152.0 vs 155.0us with the 1/4-on-DVE split.
PROD_CONFIG = dict(
    variant="v10s",
    g_blocks=8,
    bufs=(6, 3, 4),
    store_engine=None,
    encode_dve_every=None,
)


def _get_nc():
    key = (TOK_PER_CORE, N_CORES)
    if key not in _NC_CACHE:
        _NC_CACHE[key] = build_nc(**PROD_CONFIG)
    return _NC_CACHE[key]


def make_in_maps(a, b, n_cores=N_CORES, n_tok_core=TOK_PER_CORE):
    iota = np.ascontiguousarray(
        np.broadcast_to(np.arange(256, dtype=np.float32), (P, 256))
    )
    in_maps = []
    for c in range(n_cores):
        sl = slice(c * n_tok_core, (c + 1) * n_tok_core)
        in_maps.append(
            {
                "a": np.ascontiguousarray(a[:, sl]),
                "b": np.ascontiguousarray(b[:, sl]),
                "iota": iota,
            }
        )
    return in_maps


def kernel(**inputs):
    a = np.asarray(inputs["a"], dtype=np.float32)
    b = np.asarray(inputs["b"], dtype=np.float32)
    nc = _get_nc()
    res = run_bass_kernel_spmd(nc, make_in_maps(a, b), core_ids=list(range(N_CORES)))
    return np.concatenate([res.results[c]["o"] for c in range(N_CORES)], axis=1)



# revision 18
# speedup vs baseline: 1.0094x; 1.0094x over previous
"""Trainium2 Bass kernel for nn_C4TransformerVM (neural-ALU 4-byte one-hot adder).

The reference module performs 32-bit addition over one-hot byte encodings via
LUT matmuls + softmax(TEMP=100). With exact one-hot inputs the softmax output
is one-hot to within ~4e-44 (fp32 subnormals), so an exact
decode -> integer ripple-carry add -> one-hot encode pipeline matches the
reference to ~1e-43 relative error while touching each input/output element
exactly once (memory-bound regime).

Per core (pure data parallel over the batch, 4096 tokens each; production
variant "v10s" (= v9 + half-tile stores: each byte's 1MB store is issued as
two 512KB halves so the first fires 4 encode-chunks earlier), ~156us/core.
Roofline: per-NC HBM bandwidth is ~342 GB/s with reads and writes SHARING
that budget (measured reads-only 98us/32MB, writes-only 49us/16MB), so the
48MB-of-I/O floor is ~147us arithmetic / ~149.5us best measured DMA-only
structure (denc probe); the kernel runs at ~96% of that. Residual ~6us is
decode coupling: DVE (dots + carry + 1/4 of encode, ~120us/rep) paces
load-buffer release):
  pipeline: per 1024-token group, the four bytes are processed as a rippled
          byte pipeline (load_i -> dot_i -> carry-step_i -> encode_i ->
          store_i), so the first store launches ~10us into the group instead
          of after all four byte loads, keeping the store ring busy through
          the per-iteration engine-barrier refill
  decode: fused multiply-by-iota + sum-reduce (scalar_tensor_tensor
          accum_out) directly on each loaded one-hot tile (DVE; one ALU pass
          per input element, which is the floor) -> exact fp32 byte values
  carry:  batched ripple-carry on [128, G] value tiles (DVE, tiny)
  encode: d = |iota - r| via ACT Abs with per-partition bias into PSUM, then
          one-hot = Relu(1-d) on ACT (3 of 4 chunks) or iota==r on DVE
          (every 4th chunk, balancing engine load; all-ACT encode measured
          ~2us WORSE in accurate head-to-heads)
  stores: issued from GPSIMD's SWDGE ring so they cannot head-of-line block
          loads on the sync-engine HWDGE ring (only SP/ACT/GPSIMD can issue
          DMAs; GPSIMD tensor compute is pathologically slow - DMA only)
  layout: tokens -> (partition, chunk) with consecutive tokens per partition,
          so every 1MB DMA is 8KB-contiguous per partition
Rejected with measurements: 2MB/4MB DMAs (no change), loads spread over 2-3
DGE rings (worse), single/quarter stores (worse), scalar-ring stores (worse),
all-ACT encode (worse), barrier loop reset (worse), g=16/32 tiles (worse).
"""

from contextlib import ExitStack

import numpy as np

import concourse.bacc as bacc
import concourse.tile as tile
from concourse import mybir
from concourse.bass_utils import run_bass_kernel_spmd

N_CORES = 8
N_TOK = 32768
P = 128
TOK_PER_CORE = N_TOK // N_CORES  # 4096


def build_nc(
    n_tok_core=TOK_PER_CORE,
    g_blocks=8,
    n_cores=N_CORES,
    n_reps=None,
    variant="full",
    scratch_io=False,
    bufs=None,
    store_engine=None,
    encode_dve_every=4,
    staggered=True,
):
    """Build + compile the per-core Bass program (SPMD: same on all cores).

    n_reps: if set, wrap the whole body in a runtime For_i loop that re-executes
    it n_reps times (for on-device timing via wall-clock differencing).
    variant: "full" | "dmaonly" (loads+stores, no compute) | "noenc"
    (decode+carry but store s instead of encoding) | "acte" (encode on ACT).
    """
    pool_hint = variant == "v10p"
    if pool_hint:
        variant = "v10s"
    nb = n_tok_core // P  # 128-token blocks per core
    g = min(g_blocks, nb)  # blocks per pipeline group
    assert nb % g == 0

    f32 = mybir.dt.float32
    op = mybir.AluOpType

    nc = bacc.Bacc(
        "TRN2",
        target_bir_lowering=False,
        debug=False,
        num_devices=n_cores,
    )
    _store_eng = lambda: {  # noqa: E731 - resolved after nc exists
        None: nc.gpsimd,
        "gpsimd": nc.gpsimd,
        "scalar": nc.scalar,
        "sync": nc.sync,
    }[store_engine]
    if scratch_io:
        # timing-only build: full-size DRAM buffers are device scratch so each
        # exec ships only iota in / a tiny tile out over the host tunnel
        a_d = nc.dram_tensor("a", [4, n_tok_core, 256], f32, kind="Internal").ap()
        b_d = nc.dram_tensor("b", [4, n_tok_core, 256], f32, kind="Internal").ap()
        o_d = nc.dram_tensor("o", [4, n_tok_core, 256], f32, kind="Internal").ap()
        t_d = nc.dram_tensor("t", [P, 256], f32, kind="ExternalOutput").ap()
    else:
        a_d = nc.dram_tensor("a", [4, n_tok_core, 256], f32, kind="ExternalInput").ap()
        b_d = nc.dram_tensor("b", [4, n_tok_core, 256], f32, kind="ExternalInput").ap()
        o_d = nc.dram_tensor("o", [4, n_tok_core, 256], f32, kind="ExternalOutput").ap()
    iota_d = nc.dram_tensor("iota", [P, 256], f32, kind="ExternalInput").ap()

    # scale buffer counts down for bigger tiles so SBUF (~24MB) still fits
    io_b, work_b, enc_b = (3, 2, 3) if g >= 16 else (6, 3, 4)
    if variant.startswith("v8"):
        io_b, work_b, enc_b = (7, 2, 5) if g == 8 else (12, 2, 8)
    if bufs is not None:
        io_b, work_b, enc_b = bufs
    with tile.TileContext(nc) as tc, ExitStack() as ctx:
        const = ctx.enter_context(tc.tile_pool(name="const", bufs=1))
        io = ctx.enter_context(tc.tile_pool(name="io", bufs=io_b))
        work = ctx.enter_context(tc.tile_pool(name="work", bufs=work_b))
        vals = ctx.enter_context(tc.tile_pool(name="vals", bufs=3))
        enc = ctx.enter_context(tc.tile_pool(name="enc", bufs=enc_b))
        psum = ctx.enter_context(tc.tile_pool(name="psum", bufs=6, space="PSUM"))

        iota_t = const.tile([P, 256], f32)
        nc.sync.dma_start(iota_t[:], iota_d)
        if scratch_io:
            nc.sync.dma_start(t_d, iota_t[:])

        rep_ctx = (
            tc.For_i(
                0,
                n_reps,
                1,
                staggered_reset=staggered,
                hint_engines=(
                    mybir.EngineType.DVE,
                    mybir.EngineType.Activation,
                    mybir.EngineType.SP,
                )
                + ((mybir.EngineType.Pool,) if pool_hint else ()),
            )
            if n_reps is not None
            else None
        )
        if rep_ctx is not None:
            ctx.enter_context(rep_ctx)

        for gi in range(nb // g):
            tok0 = gi * g * P
            ntok_g = g * P

            if variant in ("dmar", "dmaw"):
                # directional probes: dmar = loads only, dmaw = stores only
                for i in range(4):
                    if variant == "dmar":
                        for src, tag in ((a_d, "a"), (b_d, "b")):
                            v = src[i, tok0 : tok0 + ntok_g, :].rearrange(
                                "(p c) f -> p c f", p=P
                            )
                            t_ = io.tile([P, g, 256], f32, tag=tag)
                            nc.sync.dma_start(t_[:], v)
                    else:
                        o_t = enc.tile([P, g, 256], f32, tag="o")
                        nc.vector.memset(o_t[:], 0.0)
                        o_v = o_d[i, tok0 : tok0 + ntok_g, :].rearrange(
                            "(p c) f -> p c f", p=P
                        )
                        nc.gpsimd.dma_start(o_v, o_t[:])
                continue

            if variant.startswith("dma"):
                bf16 = mybir.dt.bfloat16
                if variant == "dma2":
                    load_engs, store_engs = (nc.sync, nc.sync), (nc.scalar,)
                elif variant == "dma3":
                    load_engs, store_engs = (nc.sync, nc.sync), (nc.gpsimd,)
                elif variant == "dma4":
                    load_engs, store_engs = (nc.sync, nc.scalar), (nc.gpsimd,)
                elif variant == "dma5":
                    load_engs, store_engs = (nc.sync, nc.scalar, nc.gpsimd), (
                        nc.scalar,
                        nc.gpsimd,
                        nc.sync,
                    )
                elif variant in ("dmabf", "dmab2"):
                    load_engs, store_engs = (nc.sync, nc.sync), (nc.gpsimd,)
                else:
                    load_engs, store_engs = (nc.sync, nc.sync), (nc.sync,)
                strided = variant in ("dmabf", "dmab2")
                # [4, n, 256] f32 viewed as [4, n, 256, 2] bf16; [..., 1] = upper halves
                if strided:
                    a_hi = a_d.bitcast(bf16).rearrange("i n (f two) -> i n f two", two=2)
                    b_hi = b_d.bitcast(bf16).rearrange("i n (f two) -> i n f two", two=2)
                    o_hi = o_d.bitcast(bf16).rearrange("i n (f two) -> i n f two", two=2)
                for i in range(4):
                    if strided:
                        a_v = a_hi[i, tok0 : tok0 + ntok_g, :, 1].rearrange(
                            "(p c) f -> p c f", p=P
                        )
                        b_v = b_hi[i, tok0 : tok0 + ntok_g, :, 1].rearrange(
                            "(p c) f -> p c f", p=P
                        )
                        a_t = io.tile([P, g, 256], bf16, tag="a")
                        b_t = io.tile([P, g, 256], bf16, tag="b")
                    else:
                        a_v = a_d[i, tok0 : tok0 + ntok_g, :].rearrange(
                            "(p c) f -> p c f", p=P
                        )
                        b_v = b_d[i, tok0 : tok0 + ntok_g, :].rearrange(
                            "(p c) f -> p c f", p=P
                        )
                        a_t = io.tile([P, g, 256], f32, tag="a")
                        b_t = io.tile([P, g, 256], f32, tag="b")
                    load_engs[(2 * i) % len(load_engs)].dma_start(a_t[:], a_v)
                    load_engs[(2 * i + 1) % len(load_engs)].dma_start(b_t[:], b_v)
                    st = store_engs[i % len(store_engs)]
                    if variant == "dmab2":
                        # strided upper-half store (8MiB instead of 16MiB)
                        o_v = o_hi[i, tok0 : tok0 + ntok_g, :, 1].rearrange(
                            "(p c) f -> p c f", p=P
                        )
                        st.dma_start(o_v, a_t[:])
                    elif variant == "dmabf":
                        o_t = enc.tile([P, g, 256], f32, tag="o")
                        nc.vector.memset(o_t[:], 0.0)
                        o_v = o_d[i, tok0 : tok0 + ntok_g, :].rearrange(
                            "(p c) f -> p c f", p=P
                        )
                        st.dma_start(o_v, o_t[:])
                    else:
                        o_v = o_d[i, tok0 : tok0 + ntok_g, :].rearrange(
                            "(p c) f -> p c f", p=P
                        )
                        st.dma_start(o_v, a_t[:])
                continue

            if variant in ("dcmp", "denc"):
                # probes: dma3 traffic + one engine's compute, no cross deps
                ta = vals.tile([P, 4, g], f32, tag="ta")
                trash_d = work.tile([P, 256], f32, tag="trash_d")
                for i in range(4):
                    a_v = a_d[i, tok0 : tok0 + ntok_g, :].rearrange(
                        "(p c) f -> p c f", p=P
                    )
                    b_v = b_d[i, tok0 : tok0 + ntok_g, :].rearrange(
                        "(p c) f -> p c f", p=P
                    )
                    a_t = io.tile([P, g, 256], f32, tag="a")
                    b_t = io.tile([P, g, 256], f32, tag="b")
                    nc.sync.dma_start(a_t[:], a_v)
                    nc.sync.dma_start(b_t[:], b_v)
                    if variant == "dcmp":
                        for c in range(g):
                            for src in (a_t, b_t):
                                nc.vector.scalar_tensor_tensor(
                                    out=trash_d[:],
                                    in0=src[:, c, :],
                                    scalar=1.0,
                                    in1=iota_t[:],
                                    op0=op.mult,
                                    op1=op.mult,
                                    accum_out=ta[:, i, c % 4 : c % 4 + 1],
                                )
                        st_t = a_t
                    else:
                        o_t = enc.tile([P, g, 256], f32, tag="o")
                        for c in range(g):
                            idx = i * g + c
                            r_ap = iota_t[:, i : i + 1]
                            if idx % 4 != 3:
                                d_t = psum.tile([P, 256], f32, tag="d")
                                nc.scalar.activation(
                                    out=d_t[:],
                                    in_=iota_t[:],
                                    func=mybir.ActivationFunctionType.Abs,
                                    bias=r_ap,
                                    scale=-1.0,
                                )
                                nc.scalar.activation(
                                    out=o_t[:, c, :],
                                    in_=d_t[:],
                                    func=mybir.ActivationFunctionType.Relu,
                                    bias=1.0,
                                    scale=-1.0,
                                )
                            else:
                                nc.vector.tensor_scalar(
                                    out=o_t[:, c, :],
                                    in0=iota_t[:],
                                    scalar1=r_ap,
                                    scalar2=None,
                                    op0=op.is_equal,
                                )
                        st_t = o_t
                    o_v = o_d[i, tok0 : tok0 + ntok_g, :].rearrange(
                        "(p c) f -> p c f", p=P
                    )
                    _store_eng().dma_start(o_v, st_t[:])
                continue

            if variant in ("v10", "v10s", "v10q"):
                # v9 + split stores: each byte's store is issued in pieces as
                # soon as the covering encode chunks are done (v10s halves,
                # v10q quarters); v10 also splits loads (measured worse)
                split_loads = variant == "v10"
                n_pieces = 4 if variant == "v10q" else 2
                piece = g // n_pieces
                h = g // 2
                trash_d = work.tile([P, 256], f32, tag="trash_d")
                prev_c = None
                for i in range(4):
                    a_v = a_d[i, tok0 : tok0 + ntok_g, :].rearrange(
                        "(p c) f -> p c f", p=P
                    )
                    b_v = b_d[i, tok0 : tok0 + ntok_g, :].rearrange(
                        "(p c) f -> p c f", p=P
                    )
                    a_t = io.tile([P, g, 256], f32, tag="a")
                    b_t = io.tile([P, g, 256], f32, tag="b")
                    if split_loads:
                        for t_, v_ in ((a_t, a_v), (b_t, b_v)):
                            nc.sync.dma_start(t_[:, :h, :], v_[:, :h, :])
                            nc.sync.dma_start(t_[:, h:, :], v_[:, h:, :])
                    else:
                        nc.sync.dma_start(a_t[:], a_v)
                        nc.sync.dma_start(b_t[:], b_v)
                    ta = vals.tile([P, g], f32, tag="ta")
                    tb = vals.tile([P, g], f32, tag="tb")
                    for c in range(g):
                        for src, dst in ((a_t, ta), (b_t, tb)):
                            nc.vector.scalar_tensor_tensor(
                                out=trash_d[:],
                                in0=src[:, c, :],
                                scalar=1.0,
                                in1=iota_t[:],
                                op0=op.mult,
                                op1=op.mult,
                                accum_out=dst[:, c : c + 1],
                            )
                    t_i = vals.tile([P, g], f32, tag="t")
                    nc.vector.tensor_add(t_i[:], ta[:], tb[:])
                    if prev_c is not None:
                        nc.vector.tensor_add(t_i[:], t_i[:], prev_c)
                    c_i = vals.tile([P, g], f32, tag="c")
                    nc.vector.tensor_scalar(
                        out=c_i[:],
                        in0=t_i[:],
                        scalar1=255.5,
                        scalar2=None,
                        op0=op.is_gt,
                    )
                    r_i = vals.tile([P, g], f32, tag="r")
                    nc.vector.scalar_tensor_tensor(
                        out=r_i[:],
                        in0=c_i[:],
                        scalar=-256.0,
                        in1=t_i[:],
                        op0=op.mult,
                        op1=op.add,
                    )
                    prev_c = c_i[:]

                    o_t = enc.tile([P, g, 256], f32, tag="o")
                    o_v = o_d[i, tok0 : tok0 + ntok_g, :].rearrange(
                        "(p c) f -> p c f", p=P
                    )
                    for c in range(g):
                        r_ap = r_i[:, c : c + 1]
                        if (
                            encode_dve_every is None
                            or c % encode_dve_every != encode_dve_every - 1
                        ):
                            d_t = psum.tile([P, 256], f32, tag="d")
                            nc.scalar.activation(
                                out=d_t[:],
                                in_=iota_t[:],
                                func=mybir.ActivationFunctionType.Abs,
                                bias=r_ap,
                                scale=-1.0,
                            )
                            nc.scalar.activation(
                                out=o_t[:, c, :],
                                in_=d_t[:],
                                func=mybir.ActivationFunctionType.Relu,
                                bias=1.0,
                                scale=-1.0,
                            )
                        else:
                            nc.vector.tensor_scalar(
                                out=o_t[:, c, :],
                                in0=iota_t[:],
                                scalar1=r_ap,
                                scalar2=None,
                                op0=op.is_equal,
                            )
                        if (c + 1) % piece == 0 and c != g - 1:
                            lo_c = (c + 1) - piece
                            _store_eng().dma_start(
                                o_v[:, lo_c : c + 1, :], o_t[:, lo_c : c + 1, :]
                            )
                    _store_eng().dma_start(
                        o_v[:, g - piece :, :], o_t[:, g - piece :, :]
                    )
                continue

            if variant == "v9":
                # byte-interleaved pipeline: each byte's store launches as soon
                # as its own loads + dots + carry step + encode finish, so the
                # store ring starts ~10us into each group instead of ~30us
                trash_d = work.tile([P, 256], f32, tag="trash_d")
                prev_c = None
                for i in range(4):
                    a_v = a_d[i, tok0 : tok0 + ntok_g, :].rearrange(
                        "(p c) f -> p c f", p=P
                    )
                    b_v = b_d[i, tok0 : tok0 + ntok_g, :].rearrange(
                        "(p c) f -> p c f", p=P
                    )
                    a_t = io.tile([P, g, 256], f32, tag="a")
                    b_t = io.tile([P, g, 256], f32, tag="b")
                    nc.sync.dma_start(a_t[:], a_v)
                    nc.sync.dma_start(b_t[:], b_v)
                    ta = vals.tile([P, g], f32, tag="ta")
                    tb = vals.tile([P, g], f32, tag="tb")
                    for c in range(g):
                        nc.vector.scalar_tensor_tensor(
                            out=trash_d[:],
                            in0=a_t[:, c, :],
                            scalar=1.0,
                            in1=iota_t[:],
                            op0=op.mult,
                            op1=op.mult,
                            accum_out=ta[:, c : c + 1],
                        )
                        nc.vector.scalar_tensor_tensor(
                            out=trash_d[:],
                            in0=b_t[:, c, :],
                            scalar=1.0,
                            in1=iota_t[:],
                            op0=op.mult,
                            op1=op.mult,
                            accum_out=tb[:, c : c + 1],
                        )
                    t_i = vals.tile([P, g], f32, tag="t")
                    nc.vector.tensor_add(t_i[:], ta[:], tb[:])
                    if prev_c is not None:
                        nc.vector.tensor_add(t_i[:], t_i[:], prev_c)
                    c_i = vals.tile([P, g], f32, tag="c")
                    nc.vector.tensor_scalar(
                        out=c_i[:],
                        in0=t_i[:],
                        scalar1=255.5,
                        scalar2=None,
                        op0=op.is_gt,
                    )
                    r_i = vals.tile([P, g], f32, tag="r")
                    nc.vector.scalar_tensor_tensor(
                        out=r_i[:],
                        in0=c_i[:],
                        scalar=-256.0,
                        in1=t_i[:],
                        op0=op.mult,
                        op1=op.add,
                    )
                    prev_c = c_i[:]

                    o_t = enc.tile([P, g, 256], f32, tag="o")
                    for c in range(g):
                        r_ap = r_i[:, c : c + 1]
                        if c % 4 != 3:
                            d_t = psum.tile([P, 256], f32, tag="d")
                            nc.scalar.activation(
                                out=d_t[:],
                                in_=iota_t[:],
                                func=mybir.ActivationFunctionType.Abs,
                                bias=r_ap,
                                scale=-1.0,
                            )
                            nc.scalar.activation(
                                out=o_t[:, c, :],
                                in_=d_t[:],
                                func=mybir.ActivationFunctionType.Relu,
                                bias=1.0,
                                scale=-1.0,
                            )
                        else:
                            nc.vector.tensor_scalar(
                                out=o_t[:, c, :],
                                in0=iota_t[:],
                                scalar1=r_ap,
                                scalar2=None,
                                op0=op.is_equal,
                            )
                    o_v = o_d[i, tok0 : tok0 + ntok_g, :].rearrange(
                        "(p c) f -> p c f", p=P
                    )
                    _store_eng().dma_start(o_v, o_t[:])
                continue

            if variant.startswith("v8"):
                # decode: one fused mult+reduce pass per input element, split
                # DVE/GPSIMD; encode: one-hot via is_equal (DVE/GPS) or
                # Abs->Relu through PSUM (ACT); stores from gpsimd SWDGE
                store_eng = nc.scalar if variant == "v8s" else nc.gpsimd
                ta = vals.tile([P, 4, g], f32, tag="ta")
                tb = vals.tile([P, 4, g], f32, tag="tb")
                trash_d = work.tile([P, 256], f32, tag="trash_d")
                trash_g = work.tile([P, 256], f32, tag="trash_g")
                for i in range(4):
                    a_v = a_d[i, tok0 : tok0 + ntok_g, :].rearrange(
                        "(p c) f -> p c f", p=P
                    )
                    b_v = b_d[i, tok0 : tok0 + ntok_g, :].rearrange(
                        "(p c) f -> p c f", p=P
                    )
                    a_t = io.tile([P, g, 256], f32, tag="a")
                    b_t = io.tile([P, g, 256], f32, tag="b")
                    nc.sync.dma_start(a_t[:], a_v)
                    nc.sync.dma_start(b_t[:], b_v)
                    for c in range(g):
                        for src, dst in ((a_t, ta), (b_t, tb)):
                            nc.vector.scalar_tensor_tensor(
                                out=trash_d[:],
                                in0=src[:, c, :],
                                scalar=1.0,
                                in1=iota_t[:],
                                op0=op.mult,
                                op1=op.mult,
                                accum_out=dst[:, i, c : c + 1],
                            )
                t0 = vals.tile([P, 4, g], f32, tag="t0")
                nc.vector.tensor_add(t0[:], ta[:], tb[:])

                # ripple carry (all small [P, g] DVE ops)
                r = vals.tile([P, 4, g], f32, tag="r")
                c_t = vals.tile([P, 4, g], f32, tag="c")
                prev = None
                for i in range(4):
                    if prev is None:
                        t_i = t0[:, i, :]
                    else:
                        t_tile = vals.tile([P, g], f32, tag="t")
                        nc.vector.tensor_add(t_tile[:], t0[:, i, :], prev)
                        t_i = t_tile[:]
                    nc.vector.tensor_scalar(
                        out=c_t[:, i, :],
                        in0=t_i,
                        scalar1=255.5,
                        scalar2=None,
                        op0=op.is_gt,
                    )
                    nc.vector.scalar_tensor_tensor(
                        out=r[:, i, :],
                        in0=c_t[:, i, :],
                        scalar=-256.0,
                        in1=t_i,
                        op0=op.mult,
                        op1=op.add,
                    )
                    prev = c_t[:, i, :]

                for i in range(4):
                    o_t = enc.tile([P, g, 256], f32, tag="o")
                    for c in range(g):
                        idx = i * g + c
                        r_ap = r[:, i, c : c + 1]
                        if idx % 4 != 3:
                            d_t = psum.tile([P, 256], f32, tag="d")
                            nc.scalar.activation(
                                out=d_t[:],
                                in_=iota_t[:],
                                func=mybir.ActivationFunctionType.Abs,
                                bias=r_ap,
                                scale=-1.0,
                            )
                            nc.scalar.activation(
                                out=o_t[:, c, :],
                                in_=d_t[:],
                                func=mybir.ActivationFunctionType.Relu,
                                bias=1.0,
                                scale=-1.0,
                            )
                        else:
                            nc.vector.tensor_scalar(
                                out=o_t[:, c, :],
                                in0=iota_t[:],
                                scalar1=r_ap,
                                scalar2=None,
                                op0=op.is_equal,
                            )
                    o_v = o_d[i, tok0 : tok0 + ntok_g, :].rearrange(
                        "(p c) f -> p c f", p=P
                    )
                    store_eng.dma_start(o_v, o_t[:])
                continue

            # t0[:, i, c] = av + bv for byte i, block c (value 0..510)
            t0 = vals.tile([P, 4, g], f32, tag="t0")
            for i in range(4):
                a_v = a_d[i, tok0 : tok0 + ntok_g, :].rearrange(
                    "(p c) f -> p c f", p=P
                )
                b_v = b_d[i, tok0 : tok0 + ntok_g, :].rearrange(
                    "(p c) f -> p c f", p=P
                )
                a_t = io.tile([P, g, 256], f32, tag="a")
                b_t = io.tile([P, g, 256], f32, tag="b")
                nc.sync.dma_start(a_t[:], a_v)
                nc.sync.dma_start(b_t[:], b_v)
                s_t = work.tile([P, g, 256], f32, tag="s")
                add_eng = nc.gpsimd if variant == "v3" else nc.vector
                add_eng.tensor_add(s_t[:], a_t[:], b_t[:])
                trash = work.tile([P, 256], f32, tag="trash")
                for c in range(g):
                    nc.vector.scalar_tensor_tensor(
                        out=trash[:],
                        in0=s_t[:, c, :],
                        scalar=1.0,
                        in1=iota_t[:],
                        op0=op.mult,
                        op1=op.mult,
                        accum_out=t0[:, i, c : c + 1],
                    )
                if variant == "noenc":
                    o_v = o_d[i, tok0 : tok0 + ntok_g, :].rearrange(
                        "(p c) f -> p c f", p=P
                    )
                    nc.sync.dma_start(o_v, s_t[:])

            if variant == "noenc":
                continue

            # ripple carry: t_i = t0_i + c_i ; c_{i+1} = t_i > 255 ; r_i = t_i - 256*c_{i+1}
            r = vals.tile([P, 4, g], f32, tag="r")
            c_t = vals.tile([P, 4, g], f32, tag="c")
            prev = None
            for i in range(4):
                if prev is None:
                    t_i = t0[:, i, :]
                else:
                    t_tile = vals.tile([P, g], f32, tag="t")
                    nc.vector.tensor_add(t_tile[:], t0[:, i, :], prev)
                    t_i = t_tile[:]
                nc.vector.tensor_scalar(
                    out=c_t[:, i, :], in0=t_i, scalar1=255.5, scalar2=None, op0=op.is_gt
                )
                if variant not in ("full",):
                    # rn = 256*c - t = -r  (bias for ACT-side |iota - r|)
                    nc.vector.scalar_tensor_tensor(
                        out=r[:, i, :],
                        in0=c_t[:, i, :],
                        scalar=256.0,
                        in1=t_i,
                        op0=op.mult,
                        op1=op.subtract,
                    )
                else:
                    nc.vector.scalar_tensor_tensor(
                        out=r[:, i, :],
                        in0=c_t[:, i, :],
                        scalar=-256.0,
                        in1=t_i,
                        op0=op.mult,
                        op1=op.add,
                    )
                prev = c_t[:, i, :]

            for i in range(4):
                o_t = enc.tile([P, g, 256], f32, tag="o")
                if variant not in ("full",):
                    for c in range(g):
                        d_t = psum.tile([P, 256], f32, tag="d")
                        nc.scalar.activation(
                            out=d_t[:],
                            in_=iota_t[:],
                            func=mybir.ActivationFunctionType.Abs,
                            bias=r[:, i, c : c + 1],
                            scale=1.0,
                        )
                        if (
                            variant == "v2"
                            or (variant in ("v4", "v6", "v7") and c % 4 == 0)
                            or (variant == "v6b" and c % 2 == 0)
                        ):
                            nc.vector.tensor_scalar(
                                out=o_t[:, c, :],
                                in0=d_t[:],
                                scalar1=0.5,
                                scalar2=None,
                                op0=op.is_lt,
                            )
                        else:
                            nc.scalar.activation(
                                out=o_t[:, c, :],
                                in_=d_t[:],
                                func=mybir.ActivationFunctionType.Relu,
                                bias=1.0,
                                scale=-1.0,
                            )
                else:
                    for c in range(g):
                        nc.gpsimd.tensor_scalar(
                            out=o_t[:, c, :],
                            in0=iota_t[:],
                            scalar1=r[:, i, c : c + 1],
                            scalar2=None,
                            op0=op.is_equal,
                        )
                o_v = o_d[i, tok0 : tok0 + ntok_g, :].rearrange(
                    "(p c) f -> p c f", p=P
                )
                store_eng = (
                    nc.gpsimd
                    if variant in ("v5", "v6", "v6b")
                    else (nc.scalar if variant == "v7" else nc.sync)
                )
                store_eng.dma_start(o_v, o_t[:])

    nc.compile()
    return nc


_NC_CACHE = {}

# production build configuration (test.py's device timer uses the same dict so
# the printed HW exec time is measured on the identical per-rep body).
# Under the low-noise cached-runner estimator, all reasonable configs measure
# 155-160us (batch drift +-3us): the original 1/4-on-DVE encode split beat
# all-ACT encode in both accurate head-to-heads, and io bufs 7 (3.5 load pairs
# in flight) edged 6 in the one same-batch comparison.
PROD_CONFIG = dict(
    variant="v10s",
    g_blocks=8,
    bufs=(7, 3, 4),
    store_engine=None,
    encode_dve_every=4,
)


def _get_nc():
    key = (TOK_PER_CORE, N_CORES)
    if key not in _NC_CACHE:
        _NC_CACHE[key] = build_nc(**PROD_CONFIG)
    return _NC_CACHE[key]


def make_in_maps(a, b, n_cores=N_CORES, n_tok_core=TOK_PER_CORE):
    iota = np.ascontiguousarray(
        np.broadcast_to(np.arange(256, dtype=np.float32), (P, 256))
    )
    in_maps = []
    for c in range(n_cores):
        sl = slice(c * n_tok_core, (c + 1) * n_tok_core)
        in_maps.append(
            {
                "a": np.ascontiguousarray(a[:, sl]),
                "b": np.ascontiguousarray(b[:, sl]),
                "iota": iota,
            }
        )
    return in_maps


def kernel(**inputs):
    a = np.asarray(inputs["a"], dtype=np.float32)
    b = np.asarray(inputs["b"], dtype=np.float32)
    nc = _get_nc()
    res = run_bass_kernel_spmd(nc, make_in_maps(a, b), core_ids=list(range(N_CORES)))
    return np.concatenate([res.results[c]["o"] for c in range(N_CORES)], axis=1)

